# revision 5
# baseline (speedup 1.0000x reference)
"""CrossCompressUnit kernel for TRN2 (8 NeuronCores, data-parallel over batch).

Math (collapsing the [B,D,D] outer product analytically):
    s1[b] = e[b,:] . w_vv      s2[b] = v[b,:] . w_ev
    s3[b] = e[b,:] . w_ve      s4[b] = v[b,:] . w_ee
    v_out[b,:] = v[b,:]*s1[b] + e[b,:]*s2[b] + b_vv
    e_out[b,:] = v[b,:]*s3[b] + e[b,:]*s4[b] + b_ee

Per-core plan (shard = 1024 rows, 8 chunks of [128 batch, 128 d]):
  - PE transposes each chunk (fp32 transpose-mode matmul vs identity), then a
    tiny matmul against packed weight pairs produces the four per-row scalars
    in batch-major [128, 4] layout (exactly the per-partition scalar layout the
    DVE tensor_scalar ops need).
  - ScalarE moves PSUM results to SBUF.
  - VectorE computes   t2 = e*s2 + b_vv   and   v_out = v*s1 + t2   (fused
    scalar_tensor_tensor), similarly for e_out with the second add on GpSimd.
  - DMAs are whole-shard strided transfers on the sync sequencer.

All constants (weight pairs, broadcast biases, identity) are packed into one
[128, 134] "aux" input so they arrive in a single DMA; one warmup op per
compute engine syncs that engine on the aux/input DMAs up front, keeping every
PE instruction at <=1 sync wait (walrus limit on Matmult/LDWEIGHTS).
"""

import sys

if "/opt/trn_rl_repo" not in sys.path:
    sys.path.insert(0, "/opt/trn_rl_repo")

from contextlib import ExitStack

import numpy as np

import concourse.bass as bass
import concourse.tile as tile
from concourse import bacc
from concourse import mybir
from concourse.bass_utils import run_bass_kernel_spmd

N_CORES = 8
B, D = 8192, 128
SHARD = B // N_CORES  # 1024 rows per core
NCHUNK = SHARD // 128  # 8 chunks of 128 rows

# aux layout (columns)
AUX_WV = 0   # [w_ev | w_ee] -> s2, s4
AUX_WE = 2   # [w_vv | w_ve] -> s1, s3
AUX_BVV = 4
AUX_BEE = 5
AUX_EYE = 6
AUX_COLS = 6 + D

F32 = mybir.dt.float32
ALU = mybir.AluOpType

_CACHE: dict = {}


def _build_program() -> bass.Bass:
    nc = bacc.Bacc(
        "TRN2", target_bir_lowering=False, debug=False, num_devices=N_CORES
    )

    v_d = nc.dram_tensor("v", (SHARD, D), F32, kind="ExternalInput").ap()
    e_d = nc.dram_tensor("e", (SHARD, D), F32, kind="ExternalInput").ap()
    aux_d = nc.dram_tensor("aux", (D, AUX_COLS), F32, kind="ExternalInput").ap()
    vo_d = nc.dram_tensor("v_out", (SHARD, D), F32, kind="ExternalOutput").ap()
    eo_d = nc.dram_tensor("e_out", (SHARD, D), F32, kind="ExternalOutput").ap()

    with tile.TileContext(nc) as tc, ExitStack() as ctx:
        const = ctx.enter_context(tc.tile_pool(name="const", bufs=1))
        bigio = ctx.enter_context(tc.tile_pool(name="bigio", bufs=1))
        warm = ctx.enter_context(tc.tile_pool(name="warm", bufs=1, space="PSUM"))
        psum_t = ctx.enter_context(tc.tile_pool(name="psum_t", bufs=3, space="PSUM"))
        psum_s = ctx.enter_context(tc.tile_pool(name="psum_s", bufs=3, space="PSUM"))
        sb_t = ctx.enter_context(tc.tile_pool(name="sb_t", bufs=3))
        sb_s = ctx.enter_context(tc.tile_pool(name="sb_s", bufs=4))
        tmp = ctx.enter_context(tc.tile_pool(name="tmp", bufs=3))

        aux = const.tile([D, AUX_COLS], F32)
        nc.sync.dma_start(aux[:], aux_d)
        w_v = aux[:, AUX_WV : AUX_WV + 2]
        w_e = aux[:, AUX_WE : AUX_WE + 2]
        bvv = aux[:, AUX_BVV : AUX_BVV + 1]
        bee = aux[:, AUX_BEE : AUX_BEE + 1]
        eye = aux[:, AUX_EYE : AUX_EYE + D]

        # whole-shard loads (row (n*128 + p) -> partition p, chunk n)
        v_sb = bigio.tile([128, SHARD], F32)
        e_sb = bigio.tile([128, SHARD], F32)
        vo_sb = bigio.tile([128, SHARD], F32)
        eo_sb = bigio.tile([128, SHARD], F32)

        nc.sync.dma_start(
            v_sb[:].rearrange("p (n d) -> p n d", d=D),
            v_d.rearrange("(n p) d -> p n d", p=128),
        )
        nc.sync.dma_start(
            e_sb[:].rearrange("p (n d) -> p n d", d=D),
            e_d.rearrange("(n p) d -> p n d", p=128),
        )

        # Warmups: sync each compute engine once on the const/input DMAs so
        # steady-state instructions carry at most one semaphore wait.
        wpsum = warm.tile([128, D], F32)
        nc.tensor.transpose(wpsum[:], eye, eye)
        wsb = const.tile([128, 1], F32)
        nc.vector.tensor_copy(wsb[:], aux[:, AUX_BVV : AUX_BVV + 1])
        wsb2 = const.tile([128, 1], F32)
        nc.gpsimd.tensor_copy(wsb2[:], v_sb[:, 0:1])

        for c in range(NCHUNK):
            v_c = v_sb[:, c * D : (c + 1) * D]
            e_c = e_sb[:, c * D : (c + 1) * D]

            # PE: transpose both chunks into one PSUM tile [d, b]
            p_t = psum_t.tile([128, 2 * D], F32)
            nc.tensor.transpose(p_t[:, 0:D], v_c, eye)
            nc.tensor.transpose(p_t[:, D : 2 * D], e_c, eye)
            vt_et = sb_t.tile([128, 2 * D], F32)
            nc.scalar.copy(vt_et[:], p_t[:])

            # PE: s columns [128b, 4] = [s2, s4, s1, s3]
            s_p = psum_s.tile([128, 4], F32)
            nc.tensor.matmul(s_p[:, 0:2], lhsT=vt_et[:, 0:D], rhs=w_v, start=True, stop=True)
            nc.tensor.matmul(s_p[:, 2:4], lhsT=vt_et[:, D : 2 * D], rhs=w_e, start=True, stop=True)
            s_sb = sb_s.tile([128, 4], F32)
            nc.scalar.copy(s_sb[:], s_p[:])

            s2 = s_sb[:, 0:1]
            s4 = s_sb[:, 1:2]
            s1 = s_sb[:, 2:3]
            s3 = s_sb[:, 3:4]

            vo_c = vo_sb[:, c * D : (c + 1) * D]
            eo_c = eo_sb[:, c * D : (c + 1) * D]

            # v_out = v*s1 + (e*s2 + b_vv)
            t2 = tmp.tile([128, D], F32)
            nc.vector.tensor_scalar(t2[:], e_c, s2, bvv, ALU.mult, ALU.add)
            nc.vector.scalar_tensor_tensor(vo_c, v_c, s1, t2[:], ALU.mult, ALU.add)

            # e_out = v*s3 + (e*s4 + b_ee)
            t4 = tmp.tile([128, D], F32)
            nc.vector.tensor_scalar(t4[:], e_c, s4, bee, ALU.mult, ALU.add)
            t3 = tmp.tile([128, D], F32)
            nc.vector.tensor_scalar(t3[:], v_c, s3, None, ALU.mult)
            nc.gpsimd.tensor_add(eo_c, t3[:], t4[:])

        nc.sync.dma_start(
            vo_d.rearrange("(n p) d -> p n d", p=128),
            vo_sb[:].rearrange("p (n d) -> p n d", d=D),
        )
        nc.sync.dma_start(
            eo_d.rearrange("(n p) d -> p n d", p=128),
            eo_sb[:].rearrange("p (n d) -> p n d", d=D),
        )

    nc.compile()
    return nc


def _get_program() -> bass.Bass:
    if "nc" not in _CACHE:
        _CACHE["nc"] = _build_program()
    return _CACHE["nc"]


def _make_aux(w_vv, b_vv, w_ev, w_ve, w_ee, b_ee) -> np.ndarray:
    aux = np.zeros((D, AUX_COLS), dtype=np.float32)
    aux[:, AUX_WV + 0] = w_ev
    aux[:, AUX_WV + 1] = w_ee
    aux[:, AUX_WE + 0] = w_vv
    aux[:, AUX_WE + 1] = w_ve
    aux[:, AUX_BVV] = np.float32(np.asarray(b_vv).reshape(-1)[0])
    aux[:, AUX_BEE] = np.float32(np.asarray(b_ee).reshape(-1)[0])
    aux[:, AUX_EYE : AUX_EYE + D] = np.eye(D, dtype=np.float32)
    return aux


def kernel(v, e, w_vv, b_vv, w_ev, w_ve, w_ee, b_ee, _trace=False):
    v = np.ascontiguousarray(v, dtype=np.float32)
    e = np.ascontiguousarray(e, dtype=np.float32)
    assert v.shape == (B, D) and e.shape == (B, D)

    aux = _make_aux(w_vv, b_vv, w_ev, w_ve, w_ee, b_ee)
    in_maps = []
    for i in range(N_CORES):
        sl = slice(i * SHARD, (i + 1) * SHARD)
        in_maps.append({"v": v[sl], "e": e[sl], "aux": aux})

    nc = _get_program()
    res = run_bass_kernel_spmd(
        nc, in_maps, core_ids=list(range(N_CORES)), trace=_trace
    )

    v_out = np.concatenate([r["v_out"] for r in res.results], axis=0)
    e_out = np.concatenate([r["e_out"] for r in res.results], axis=0)
    if _trace:
        _CACHE["last_results"] = res
    return (v_out, e_out)
